# revision 28
# baseline (speedup 1.0000x reference)
"""Trainium2 Bass kernel for sparse multi-head edge attention (V7).

Computation (per the nn.Module):
    Q = Fa @ Wq.T, K = Fb @ Wk.T, V = Fb @ Wv.T   (H=8 heads x 32)
    per edge e: logit[e,h] = <Q[a_e,h,:], K[b_e,h,:]> / sqrt(32)
    segmented softmax over edges per query, out = Fa + (softmax(V)) @ Wproj.T

V7 strategy (vs the V3 baseline at ~1.04ms):
  - Queries sharded 8 ways; edges routed to the owner core, packed into
    128-edge tiles per 128-query block (identical host routing to V3).
  - K is recomputed per-edge *d-transposed* (KT[d, e] = Wk-row d . Fb[b_e])
    so the per-head logit reduction runs on the TENSOR engine as 4 tiny
    matmuls against a constant one-hot head map (HMAP), producing logits in
    normal [e, h] layout.  This removes the 265us tensor_reduce from DVE
    (tensor_reduce has no DVE fast mode) and keeps exp cheap (16 el/part).
  - Qe is selected transposed the same way (QeT = Qblk.T @ one-hot), so the
    QK product is one DVE multiply [128, 2, 256] -> fp16 SBUF.
  - V is recomputed per-edge in normal layout, evacuated PSUM->SBUF fp16 by
    the Scalar engine; exp values are broadcast-expanded across the head dim
    by GPSIMD (SBUF->SBUF); the exp-weighted V multiply then runs on DVE in
    2x_1p mode (all operands packed fp16 SBUF).
  - Accumulation of [den | num] per query stays a one-hot matmul per tile.
  - 3-stage software pipeline (emit S1(i), S2(i-1), S3(i-2)) so no in-order
    engine queue holds an op whose dependency chain crosses engines within
    the same group.
  All fp8 variants were numerically rejected (softmax amplifies the ~3%
  e4m3 quantization of K/V to >2e-2 output error); everything stays fp16
  with fp32 accumulation, matching the V3 numerics (max rel ~2.7e-3).
"""

import math

import numpy as np

P = 128
H = 8
DH = 32
CDIM = 256
NA = 50000
NB = 50000
NCORES = 8
NAC = NA // NCORES          # 6250 queries per core
NBLK = (NAC + P - 1) // P   # 49 query blocks per core
NPADQ = NBLK * P            # 6272 padded queries per core
CHUNK = 2048
G = 2                       # tiles per op-batching group
SCALE = 1.0 / math.sqrt(DH)

F16 = np.float16
F32 = np.float32


def _ceil128(x):
    return (np.asarray(x) + P - 1) // P * P


def preprocess(Fa, Fb, a_idx, b_idx, Wq, Wk, Wv, Wproj):
    """Host-side sharding: returns (meta, shared_inputs, per_core_inputs)."""
    a_idx = np.asarray(a_idx).astype(np.int64)
    b_idx = np.asarray(b_idx).astype(np.int64)
    Fa = np.asarray(Fa, F32)
    Fb = np.asarray(Fb, F32)

    core = a_idx // NAC
    a_loc = a_idx - core * NAC
    blk = a_loc // P
    arel = a_loc % P

    cnt = np.bincount(core * NBLK + blk, minlength=NCORES * NBLK)
    cnt = cnt.reshape(NCORES, NBLK)
    CAP = np.maximum(_ceil128(cnt.max(axis=0)), P)
    coff = np.concatenate([[0], np.cumsum(CAP)])
    TOT = int(coff[-1])

    # rank of each edge within its (core, blk) group
    ne = a_idx.shape[0]
    gid = core * NBLK + blk
    order = np.argsort(gid, kind="stable")
    counts = np.bincount(gid, minlength=NCORES * NBLK)
    gstart = np.concatenate([[0], np.cumsum(counts)])[:-1]
    rank = np.empty(ne, np.int64)
    rank[order] = np.arange(ne) - gstart[gid[order]]
    slot = coff[blk] + rank

    Fb16 = Fb.astype(F16)
    # HMAP: one-hot head map per d-half: HMAP[u][p, h] = 1 iff h == u*4 + p//DH
    hmap = np.zeros((P, 2, H), F16)
    for u in range(2):
        hmap[np.arange(P), u, u * 4 + np.arange(P) // DH] = 1.0
    # WKT: Wk.T arranged [c-chunk partitions, chunk, half, 128]:
    #   WKT[p, t, u, d] = Wk.T[t*128+p, u*128+d]
    wkT = Wk.T.astype(F16)  # [256 c, 256 d]
    WKT = wkT.reshape(2, P, 2, P).transpose(1, 0, 2, 3).copy()  # [p, t, u, d]
    # WVT: Wv.T arranged [c-chunk partitions, chunk, 256]
    wvT = Wv.T.astype(F16)
    WVT = wvT.reshape(2, P, CDIM).transpose(1, 0, 2).copy()     # [p, t, d]
    shared = {
        "WqT": Wq.T.astype(F16).copy(),
        "WKT": WKT,
        "WVT": WVT,
        "HMAP": hmap,
        "WprojT": Wproj.T.astype(F16).copy(),
        "IDENT16": np.eye(P, dtype=F16),
    }

    per_core = []
    for m in range(NCORES):
        msk = core == m
        sl = slot[msk]
        ar = arel[msk]
        fbe = np.zeros((TOT, CDIM), F16)
        fbe[sl] = Fb16[b_idx[msk]]
        selT = np.zeros((P, TOT), F16)
        selT[ar, sl] = 1.0
        sel = np.zeros((P, TOT), F16)
        sel[sl % P, (sl // P) * P + ar] = 1.0

        FaT = np.zeros((CDIM, NPADQ), F16)
        FaT[:, :NAC] = Fa[m * NAC:(m + 1) * NAC].T.astype(F16)
        Fa_res = np.zeros((NPADQ, CDIM), F16)
        Fa_res[:NAC] = Fa[m * NAC:(m + 1) * NAC].astype(F16)
        fbet2 = fbe.T.reshape(2, P, TOT).transpose(1, 0, 2)   # [128, 2, TOT]
        per_core.append({
            "FbET": np.ascontiguousarray(fbet2),
            "SELT": selT,
            "SEL": sel,
            "FaT": FaT,
            "FaRes": Fa_res,
        })

    meta = {"CAP": CAP.astype(int), "coff": coff.astype(int), "TOT": TOT}
    return meta, shared, per_core


def build_program(meta):
    import concourse.bacc as bacc
    import concourse.mybir as mybir
    from concourse.tile import TileContext

    dt = mybir.dt
    nc = bacc.Bacc("TRN2", target_bir_lowering=False, debug=False,
                   num_devices=NCORES)

    CAP, coff, TOT = meta["CAP"], meta["coff"], meta["TOT"]
    CMAX = int(CAP.max()) // P
    AluOp = mybir.AluOpType

    FbET_t = nc.dram_tensor("FbET", [P, 2, TOT], dt.float16, kind="ExternalInput")
    SELT_t = nc.dram_tensor("SELT", [P, TOT], dt.float16, kind="ExternalInput")
    SEL_t = nc.dram_tensor("SEL", [P, TOT], dt.float16, kind="ExternalInput")
    FaT_t = nc.dram_tensor("FaT", [CDIM, NPADQ], dt.float16, kind="ExternalInput")
    FaRes_t = nc.dram_tensor("FaRes", [NPADQ, CDIM], dt.float16, kind="ExternalInput")
    WqT_t = nc.dram_tensor("WqT", [CDIM, CDIM], dt.float16, kind="ExternalInput")
    WKT_t = nc.dram_tensor("WKT", [P, 2, 2, P], dt.float16, kind="ExternalInput")
    WVT_t = nc.dram_tensor("WVT", [P, 2, CDIM], dt.float16, kind="ExternalInput")
    HMAP_t = nc.dram_tensor("HMAP", [P, 2, H], dt.float16, kind="ExternalInput")
    WprojT_t = nc.dram_tensor("WprojT", [CDIM, CDIM], dt.float16, kind="ExternalInput")
    IDENT_t = nc.dram_tensor("IDENT16", [P, P], dt.float16, kind="ExternalInput")
    OUT_t = nc.dram_tensor("OUT", [NPADQ, CDIM], dt.float32, kind="ExternalOutput")

    with TileContext(nc) as tc:
        with tc.tile_pool(name="res", bufs=1) as rpool:
            wq = rpool.tile([P, 2, CDIM], dt.float16, tag="wq")
            wkt = rpool.tile([P, 2, 2, P], dt.float16, tag="wkt")
            wvt = rpool.tile([P, 2, CDIM], dt.float16, tag="wvt")
            hmap = rpool.tile([P, 2, H], dt.float16, tag="hmap")
            wproj = rpool.tile([P, 2, CDIM], dt.float16, tag="wproj")
            ident16 = rpool.tile([P, P], dt.float16, tag="ident16")
            nc.sync.dma_start(out=wq[:, 0, :], in_=WqT_t[0:P, :])
            nc.sync.dma_start(out=wq[:, 1, :], in_=WqT_t[P:2 * P, :])
            nc.sync.dma_start(out=wkt[:], in_=WKT_t[:, :, :, :])
            nc.sync.dma_start(out=wvt[:], in_=WVT_t[:, :, :])
            nc.sync.dma_start(out=hmap[:], in_=HMAP_t[:, :, :])
            nc.sync.dma_start(out=wproj[:, 0, :], in_=WprojT_t[0:P, :])
            nc.sync.dma_start(out=wproj[:, 1, :], in_=WprojT_t[P:2 * P, :])
            nc.sync.dma_start(out=ident16[:], in_=IDENT_t[:, :])
            qres = rpool.tile([P, NBLK, CDIM], dt.float16, tag="qres")

            # ---- Phase B: edge attention ----
            # (Phase A — building Qblk[q, d] per block into qres — is folded
            # into the group pipeline one block ahead, borrowing qt-pool
            # tiles, so the edge pipeline starts immediately.)
            with tc.tile_pool(name="gat", bufs=4) as gpool, \
                 tc.tile_pool(name="bld", bufs=2) as bpool, \
                 tc.tile_pool(name="wrk", bufs=3) as wpool, \
                 tc.tile_pool(name="fin", bufs=3) as fpool, \
                 tc.tile_pool(name="psKT", bufs=2, space="PSUM") as psKT, \
                 tc.tile_pool(name="psQT", bufs=2, space="PSUM") as psQT, \
                 tc.tile_pool(name="psV", bufs=2, space="PSUM") as psV, \
                 tc.tile_pool(name="psDN", bufs=2, space="PSUM") as psDN:
                groups = []
                for j in range(NBLK):
                    Cj = int(CAP[j]) // P
                    for g0 in range(0, Cj, G):
                        groups.append((j, g0, min(G, Cj - g0), g0 + G >= Cj))

                blk = {}        # j -> dict(dn, sel, fbet, selt)

                def emit_phase_a(jj):
                    """Build qres[:, jj] (Qblk for block jj) via a borrowed
                    qt-pool tile."""
                    if jj >= NBLK:
                        return
                    ft = bpool.tile([P, 2, P], dt.float16, tag="ft")
                    nc.sync.dma_start(out=ft[:, 0, :],
                                      in_=FaT_t[0:P, jj * P:(jj + 1) * P])
                    nc.sync.dma_start(out=ft[:, 1, :],
                                      in_=FaT_t[P:2 * P, jj * P:(jj + 1) * P])
                    qa = psQT.tile([P, 2, G * P], dt.float32, tag="qt")
                    nc.tensor.matmul(qa[:, 0, :CDIM], ft[:, 0, :],
                                     wq[:, 0, :], start=True, stop=False)
                    nc.tensor.matmul(qa[:, 0, :CDIM], ft[:, 1, :],
                                     wq[:, 1, :], start=False, stop=True)
                    if jj % 2 == 0:
                        nc.scalar.copy(out=qres[:, jj, :], in_=qa[:, 0, :CDIM])
                    else:
                        with nc.allow_low_precision(reason="q evac fp16"):
                            nc.vector.tensor_scalar_add(
                                out=qres[:, jj, :], in0=qa[:, 0, :CDIM],
                                scalar1=0.0)

                def emit_finalize(j, dn_ps):
                    # dn bank layout: [0:264] accum (dead after s_sb) ->
                    # reused for proj out [0:256]; [264:280] logits region;
                    # [280:408] transpose scratch (one half at a time).
                    den = fpool.tile([P, H], dt.float32, tag="den")
                    nc.vector.tensor_scalar_max(out=den[:], in0=dn_ps[:, 0:H],
                                                scalar1=1e-30)
                    rec = fpool.tile([P, H], dt.float32, tag="rec")
                    nc.vector.reciprocal(out=rec[:], in_=den[:])
                    s_sb = fpool.tile([P, CDIM], dt.float16, tag="s_sb")
                    with nc.allow_low_precision(reason="softmax out fp16"):
                        nc.vector.tensor_tensor(
                            out=s_sb[:], in0=dn_ps[:, H:H + CDIM],
                            in1=rec[:].unsqueeze(2).to_broadcast([P, H, DH]),
                            op=AluOp.mult)
                    st_sb = fpool.tile([P, 2, P], dt.float16, tag="st_sb")
                    tsc = dn_ps[:, 280:280 + P // 2].bitcast(dt.float16)
                    nc.tensor.transpose(tsc, s_sb[:, 0:P], ident16[:])
                    nc.scalar.copy(out=st_sb[:, 0, :], in_=tsc)
                    nc.tensor.transpose(tsc, s_sb[:, P:2 * P], ident16[:])
                    nc.scalar.copy(out=st_sb[:, 1, :], in_=tsc)
                    proj = dn_ps[:, 0:CDIM]
                    nc.tensor.matmul(proj, st_sb[:, 0, :],
                                     wproj[:, 0, :], start=True, stop=False,
                                     skip_group_check=True)
                    nc.tensor.matmul(proj, st_sb[:, 1, :],
                                     wproj[:, 1, :], start=False, stop=True,
                                     skip_group_check=True)
                    fa_t = fpool.tile([P, CDIM], dt.float16, tag="fa_t")
                    nc.sync.dma_start(out=fa_t[:], in_=FaRes_t[j * P:(j + 1) * P, :])
                    res = fpool.tile([P, CDIM], dt.float32, tag="res")
                    nc.vector.tensor_tensor(out=res[:], in0=proj,
                                            in1=fa_t[:], op=AluOp.add)
                    nc.sync.dma_start(out=OUT_t[j * P:(j + 1) * P, :], in_=res[:])

                def emit_S1(g):
                    """DMA + PE: KT, QeT matmuls for group g."""
                    j, g0, gn, last = g["key"]
                    if j not in blk:
                        Cj = int(CAP[j]) // P
                        c0 = int(coff[j])
                        fbet = gpool.tile([P, 2, CMAX * P], dt.float16, tag="fbet")
                        nc.sync.dma_start(out=fbet[:, :, :Cj * P],
                                          in_=FbET_t[:, :, c0:c0 + Cj * P])
                        selt = gpool.tile([P, CMAX * P], dt.float16, tag="selt")
                        nc.sync.dma_start(out=selt[:, :Cj * P],
                                          in_=SELT_t[:, c0:c0 + Cj * P])
                        sel = gpool.tile([P, CMAX * P], dt.float16, tag="sel")
                        nc.sync.dma_start(out=sel[:, :Cj * P],
                                          in_=SEL_t[:, c0:c0 + Cj * P])
                        # dn bank hosts [0:264] accum, [264:280] per-group
                        # logits, [280:408] finalize transpose scratch.
                        dn_ps = psDN.tile([P, 2 * CDIM], dt.float32, tag="dn")
                        blk[j] = {"fbet": fbet, "selt": selt, "sel": sel,
                                  "dn": dn_ps}
                    st = blk[j]
                    j_, g0_, gn_ = j, g0, gn
                    e0, e1 = g0_ * P, (g0_ + gn_) * P
                    ew = gn_ * P
                    # KT[d, e] per d-half u: accumulate over c-chunks t
                    kt_ps = psKT.tile([P, 2, G * P], dt.float32, tag="kt")
                    for u in range(2):
                        for t in range(2):
                            nc.tensor.matmul(kt_ps[:, u, :ew],
                                             wkt[:, t, u, :],
                                             st["fbet"][:, t, e0:e1],
                                             start=(t == 0), stop=(t == 1))
                    # QeT[d, e] per d-half u: Qblk.T @ one-hot
                    qt_ps = psQT.tile([P, 2, G * P], dt.float32, tag="qt")
                    for u in range(2):
                        nc.tensor.matmul(qt_ps[:, u, :ew],
                                         qres[:, j_, u * P:(u + 1) * P],
                                         st["selt"][:, e0:e1],
                                         start=True, stop=True)
                    g["kt"], g["qt"] = kt_ps, qt_ps

                def emit_S2(g):
                    """Scalar: QeT evac + exp; DVE: prod; PE: logits + V."""
                    j, g0, gn, last = g["key"]
                    ew = gn * P
                    qt_sb = wpool.tile([P, 2, G * P], dt.float16, tag="qt_sb")
                    nc.scalar.copy(out=qt_sb[:, :, :ew], in_=g["qt"][:, :, :ew])
                    prod = wpool.tile([P, 2, G * P], dt.float16, tag="prod")
                    with nc.allow_low_precision(reason="qk product fp16"):
                        nc.vector.tensor_tensor(
                            out=prod[:, :, :ew], in0=g["kt"][:, :, :ew],
                            in1=qt_sb[:, :, :ew], op=AluOp.mult)
                    # logits land in the (now dead) kt tile region; the RAW
                    # dep on prod already orders this after prod's read.
                    lg_ps = g["kt"][:, 0, 0:G * H].rearrange(
                        "p (g h) -> p g h", h=H)
                    for gg in range(gn):
                        for u in range(2):
                            nc.tensor.matmul(
                                lg_ps[:, gg, :],
                                prod[:, u, (gg * P):(gg + 1) * P],
                                hmap[:, u, :],
                                start=(u == 0), stop=(u == 1),
                                skip_group_check=True)
                    exwv = wpool.tile([P, G, H + CDIM], dt.float16, tag="exwv")
                    nc.scalar.activation(
                        out=exwv[:, :gn, 0:H], in_=lg_ps[:, :gn, :],
                        func=mybir.ActivationFunctionType.Exp, scale=SCALE)
                    # V[e, d] per tile: accumulate over c-chunks t
                    st = blk[j]
                    v_ps = psV.tile([P, G, CDIM], dt.float32, tag="v")
                    for gg in range(gn):
                        tt = g0 + gg
                        for t in range(2):
                            nc.tensor.matmul(v_ps[:, gg, :],
                                             st["fbet"][:, t, tt * P:(tt + 1) * P],
                                             wvt[:, t, :],
                                             start=(t == 0), stop=(t == 1))
                    g["exwv"], g["v"] = exwv, v_ps

                def emit_S3(g):
                    """DVE: exwv (broadcast exp); PE: accum (+fin)."""
                    j, g0, gn, last = g["key"]
                    Cj = int(CAP[j]) // P
                    exwv = g["exwv"]
                    with nc.allow_low_precision(reason="exp-weighted V fp16"):
                        nc.vector.tensor_tensor(
                            out=exwv[:, :gn, H:H + CDIM],
                            in0=exwv[:, :gn, 0:H].unsqueeze(3).to_broadcast(
                                [P, gn, H, DH]),
                            in1=g["v"][:, :gn, :], op=AluOp.mult)
                    st = blk[j]
                    for t in range(g0, g0 + gn):
                        nc.tensor.matmul(st["dn"][:, 0:H + CDIM],
                                         st["sel"][:, t * P:(t + 1) * P],
                                         exwv[:, t - g0, :],
                                         start=(t == 0), stop=(t == Cj - 1),
                                         skip_group_check=True)
                    if last:
                        emit_finalize(j, st["dn"])
                        del blk[j]

                # Phase A upfront: Scalar evacuations split with DVE so the
                # edge pipeline's first Scalar/DVE ops aren't queued behind
                # 49 serial copies.
                for jj in range(NBLK):
                    emit_phase_a(jj)
                pend = []
                for (j, g0, gn, last) in groups:
                    g = {"key": (j, g0, gn, last)}
                    emit_S1(g)
                    if len(pend) >= 2:
                        emit_S3(pend[-2])
                    if len(pend) >= 1:
                        emit_S2(pend[-1])
                    if len(pend) >= 2:
                        pend.pop(0)
                    pend.append(g)
                if pend:
                    emit_S2(pend[-1])
                    emit_S3(pend[0])
                    if len(pend) >= 2:
                        emit_S3(pend[1])

    nc.compile()
    return nc


TRACE = False          # set by test harness for NTFF profiling
LAST_RESULT = None     # BassKernelResults of the last run (for profiling)


def kernel(**inputs):
    global LAST_RESULT
    from concourse.bass_utils import run_bass_kernel_spmd

    meta, shared, per_core = preprocess(**inputs)
    nc = build_program(meta)
    in_maps = [dict(shared, **pc) for pc in per_core]
    res = run_bass_kernel_spmd(nc, in_maps, core_ids=list(range(NCORES)),
                               trace=TRACE)
    LAST_RESULT = res
    out = np.empty((NA, CDIM), F32)
    for m in range(NCORES):
        out[m * NAC:(m + 1) * NAC] = res.results[m]["OUT"][:NAC]
    return out


# revision 30
# speedup vs baseline: 1.1047x; 1.1047x over previous
"""Trainium2 Bass kernel for sparse multi-head edge attention (V7).

Computation (per the nn.Module):
    Q = Fa @ Wq.T, K = Fb @ Wk.T, V = Fb @ Wv.T   (H=8 heads x 32)
    per edge e: logit[e,h] = <Q[a_e,h,:], K[b_e,h,:]> / sqrt(32)
    segmented softmax over edges per query, out = Fa + (softmax(V)) @ Wproj.T

V7 strategy (vs the V3 baseline at ~1.04ms):
  - Queries sharded 8 ways; edges routed to the owner core, packed into
    128-edge tiles per 128-query block (identical host routing to V3).
  - K is recomputed per-edge *d-transposed* (KT[d, e] = Wk-row d . Fb[b_e])
    so the per-head logit reduction runs on the TENSOR engine as 4 tiny
    matmuls against a constant one-hot head map (HMAP), producing logits in
    normal [e, h] layout.  This removes the 265us tensor_reduce from DVE
    (tensor_reduce has no DVE fast mode) and keeps exp cheap (16 el/part).
  - Qe is selected transposed the same way (QeT = Qblk.T @ one-hot), so the
    QK product is one DVE multiply [128, 2, 256] -> fp16 SBUF.
  - V is recomputed per-edge in normal layout, evacuated PSUM->SBUF fp16 by
    the Scalar engine; exp values are broadcast-expanded across the head dim
    by GPSIMD (SBUF->SBUF); the exp-weighted V multiply then runs on DVE in
    2x_1p mode (all operands packed fp16 SBUF).
  - Accumulation of [den | num] per query stays a one-hot matmul per tile.
  - 3-stage software pipeline (emit S1(i), S2(i-1), S3(i-2)) so no in-order
    engine queue holds an op whose dependency chain crosses engines within
    the same group.
  All fp8 variants were numerically rejected (softmax amplifies the ~3%
  e4m3 quantization of K/V to >2e-2 output error); everything stays fp16
  with fp32 accumulation, matching the V3 numerics (max rel ~2.7e-3).
"""

import math

import numpy as np

P = 128
H = 8
DH = 32
CDIM = 256
NA = 50000
NB = 50000
NCORES = 8
NAC = NA // NCORES          # 6250 queries per core
NBLK = (NAC + P - 1) // P   # 49 query blocks per core
NPADQ = NBLK * P            # 6272 padded queries per core
CHUNK = 2048
G = 2                       # tiles per op-batching group
SCALE = 1.0 / math.sqrt(DH)

F16 = np.float16
F32 = np.float32


def _ceil128(x):
    return (np.asarray(x) + P - 1) // P * P


def preprocess(Fa, Fb, a_idx, b_idx, Wq, Wk, Wv, Wproj):
    """Host-side sharding: returns (meta, shared_inputs, per_core_inputs)."""
    a_idx = np.asarray(a_idx).astype(np.int64)
    b_idx = np.asarray(b_idx).astype(np.int64)
    Fa = np.asarray(Fa, F32)
    Fb = np.asarray(Fb, F32)

    core = a_idx // NAC
    a_loc = a_idx - core * NAC
    blk = a_loc // P
    arel = a_loc % P

    cnt = np.bincount(core * NBLK + blk, minlength=NCORES * NBLK)
    cnt = cnt.reshape(NCORES, NBLK)
    CAP = np.maximum(_ceil128(cnt.max(axis=0)), P)
    coff = np.concatenate([[0], np.cumsum(CAP)])
    TOT = int(coff[-1])

    # rank of each edge within its (core, blk) group
    ne = a_idx.shape[0]
    gid = core * NBLK + blk
    order = np.argsort(gid, kind="stable")
    counts = np.bincount(gid, minlength=NCORES * NBLK)
    gstart = np.concatenate([[0], np.cumsum(counts)])[:-1]
    rank = np.empty(ne, np.int64)
    rank[order] = np.arange(ne) - gstart[gid[order]]
    slot = coff[blk] + rank

    Fb16 = Fb.astype(F16)
    # HMAP: one-hot head map per d-half: HMAP[u][p, h] = 1 iff h == u*4 + p//DH
    hmap = np.zeros((P, 2, H), F16)
    for u in range(2):
        hmap[np.arange(P), u, u * 4 + np.arange(P) // DH] = 1.0
    # WKT: Wk.T arranged [c-chunk partitions, chunk, half, 128]:
    #   WKT[p, t, u, d] = Wk.T[t*128+p, u*128+d]
    wkT = Wk.T.astype(F16)  # [256 c, 256 d]
    WKT = wkT.reshape(2, P, 2, P).transpose(1, 0, 2, 3).copy()  # [p, t, u, d]
    # WVT: Wv.T arranged [c-chunk partitions, chunk, 256]
    wvT = Wv.T.astype(F16)
    WVT = wvT.reshape(2, P, CDIM).transpose(1, 0, 2).copy()     # [p, t, d]
    shared = {
        "WqT": Wq.T.astype(F16).copy(),
        "WKT": WKT,
        "WVT": WVT,
        "HMAP": hmap,
        "WprojT": Wproj.T.astype(F16).copy(),
        "IDENT16": np.eye(P, dtype=F16),
    }

    per_core = []
    for m in range(NCORES):
        msk = core == m
        sl = slot[msk]
        ar = arel[msk]
        fbe = np.zeros((TOT, CDIM), F16)
        fbe[sl] = Fb16[b_idx[msk]]
        selT = np.zeros((P, TOT), F16)
        selT[ar, sl] = 1.0
        sel = np.zeros((P, TOT), F16)
        sel[sl % P, (sl // P) * P + ar] = 1.0

        FaT = np.zeros((CDIM, NPADQ), F16)
        FaT[:, :NAC] = Fa[m * NAC:(m + 1) * NAC].T.astype(F16)
        Fa_res = np.zeros((NPADQ, CDIM), F16)
        Fa_res[:NAC] = Fa[m * NAC:(m + 1) * NAC].astype(F16)
        fbet2 = fbe.T.reshape(2, P, TOT).transpose(1, 0, 2)   # [128, 2, TOT]
        per_core.append({
            "FbET": np.ascontiguousarray(fbet2),
            "SELT": selT,
            "SEL": sel,
            "FaT": FaT,
            "FaRes": Fa_res,
        })

    meta = {"CAP": CAP.astype(int), "coff": coff.astype(int), "TOT": TOT}
    return meta, shared, per_core


def build_program(meta):
    import concourse.bacc as bacc
    import concourse.mybir as mybir
    from concourse.tile import TileContext

    dt = mybir.dt
    nc = bacc.Bacc("TRN2", target_bir_lowering=False, debug=False,
                   num_devices=NCORES)

    CAP, coff, TOT = meta["CAP"], meta["coff"], meta["TOT"]
    CMAX = int(CAP.max()) // P
    AluOp = mybir.AluOpType

    FbET_t = nc.dram_tensor("FbET", [P, 2, TOT], dt.float16, kind="ExternalInput")
    SELT_t = nc.dram_tensor("SELT", [P, TOT], dt.float16, kind="ExternalInput")
    SEL_t = nc.dram_tensor("SEL", [P, TOT], dt.float16, kind="ExternalInput")
    FaT_t = nc.dram_tensor("FaT", [CDIM, NPADQ], dt.float16, kind="ExternalInput")
    FaRes_t = nc.dram_tensor("FaRes", [NPADQ, CDIM], dt.float16, kind="ExternalInput")
    WqT_t = nc.dram_tensor("WqT", [CDIM, CDIM], dt.float16, kind="ExternalInput")
    WKT_t = nc.dram_tensor("WKT", [P, 2, 2, P], dt.float16, kind="ExternalInput")
    WVT_t = nc.dram_tensor("WVT", [P, 2, CDIM], dt.float16, kind="ExternalInput")
    HMAP_t = nc.dram_tensor("HMAP", [P, 2, H], dt.float16, kind="ExternalInput")
    WprojT_t = nc.dram_tensor("WprojT", [CDIM, CDIM], dt.float16, kind="ExternalInput")
    IDENT_t = nc.dram_tensor("IDENT16", [P, P], dt.float16, kind="ExternalInput")
    OUT_t = nc.dram_tensor("OUT", [NPADQ, CDIM], dt.float32, kind="ExternalOutput")

    with TileContext(nc) as tc:
        with tc.tile_pool(name="res", bufs=1) as rpool:
            wq = rpool.tile([P, 2, CDIM], dt.float16, tag="wq")
            wkt = rpool.tile([P, 2, 2, P], dt.float16, tag="wkt")
            wvt = rpool.tile([P, 2, CDIM], dt.float16, tag="wvt")
            hmap = rpool.tile([P, 2, H], dt.float16, tag="hmap")
            wproj = rpool.tile([P, 2, CDIM], dt.float16, tag="wproj")
            ident16 = rpool.tile([P, P], dt.float16, tag="ident16")
            nc.sync.dma_start(out=wq[:, 0, :], in_=WqT_t[0:P, :])
            nc.sync.dma_start(out=wq[:, 1, :], in_=WqT_t[P:2 * P, :])
            nc.sync.dma_start(out=wkt[:], in_=WKT_t[:, :, :, :])
            nc.sync.dma_start(out=wvt[:], in_=WVT_t[:, :, :])
            nc.sync.dma_start(out=hmap[:], in_=HMAP_t[:, :, :])
            nc.sync.dma_start(out=wproj[:, 0, :], in_=WprojT_t[0:P, :])
            nc.sync.dma_start(out=wproj[:, 1, :], in_=WprojT_t[P:2 * P, :])
            nc.sync.dma_start(out=ident16[:], in_=IDENT_t[:, :])
            qres = rpool.tile([P, NBLK, CDIM], dt.float16, tag="qres")

            # ---- Phase A: build Q into SBUF (Qblk[q, d] per block) ----
            with tc.tile_pool(name="bldA", bufs=2) as bpoolA, \
                 tc.tile_pool(name="psA", bufs=4, space="PSUM") as psA:
                for c0 in range(0, NPADQ, CHUNK):
                    nsub = min(CHUNK, NPADQ - c0) // P
                    ft = bpoolA.tile([P, 2, CHUNK], dt.float16, tag="ft")
                    nc.sync.dma_start(out=ft[:, 0, :nsub * P],
                                      in_=FaT_t[0:P, c0:c0 + nsub * P])
                    nc.sync.dma_start(out=ft[:, 1, :nsub * P],
                                      in_=FaT_t[P:2 * P, c0:c0 + nsub * P])
                    for s in range(nsub):
                        ps = psA.tile([P, CDIM], dt.float32, tag="psA")
                        nc.tensor.matmul(ps[:], ft[:, 0, s * P:(s + 1) * P],
                                         wq[:, 0, :], start=True, stop=False)
                        nc.tensor.matmul(ps[:], ft[:, 1, s * P:(s + 1) * P],
                                         wq[:, 1, :], start=False, stop=True)
                        jj = c0 // P + s
                        if jj % 2 == 0:
                            nc.scalar.copy(out=qres[:, jj, :], in_=ps[:])
                        else:
                            with nc.allow_low_precision(reason="q evac fp16"):
                                nc.vector.tensor_scalar_add(
                                    out=qres[:, jj, :], in0=ps[:],
                                    scalar1=0.0)

            # ---- Phase B: edge attention ----
            with tc.tile_pool(name="gat", bufs=4) as gpool, \
                 tc.tile_pool(name="wrk", bufs=3) as wpool, \
                 tc.tile_pool(name="fin", bufs=3) as fpool, \
                 tc.tile_pool(name="psKT", bufs=2, space="PSUM") as psKT, \
                 tc.tile_pool(name="psQT", bufs=2, space="PSUM") as psQT, \
                 tc.tile_pool(name="psV", bufs=2, space="PSUM") as psV, \
                 tc.tile_pool(name="psDN", bufs=2, space="PSUM") as psDN:
                groups = []
                for j in range(NBLK):
                    Cj = int(CAP[j]) // P
                    for g0 in range(0, Cj, G):
                        groups.append((j, g0, min(G, Cj - g0), g0 + G >= Cj))

                blk = {}        # j -> dict(dn, sel, fbet, selt)

                def emit_finalize(j, dn_ps):
                    # dn bank layout: [0:264] accum (dead after s_sb) ->
                    # reused for proj out [0:256]; [264:280] logits region;
                    # [280:408] transpose scratch (one half at a time).
                    den = fpool.tile([P, H], dt.float32, tag="den")
                    nc.vector.tensor_scalar_max(out=den[:], in0=dn_ps[:, 0:H],
                                                scalar1=1e-30)
                    rec = fpool.tile([P, H], dt.float32, tag="rec")
                    nc.vector.reciprocal(out=rec[:], in_=den[:])
                    s_sb = fpool.tile([P, CDIM], dt.float16, tag="s_sb")
                    with nc.allow_low_precision(reason="softmax out fp16"):
                        nc.vector.tensor_tensor(
                            out=s_sb[:], in0=dn_ps[:, H:H + CDIM],
                            in1=rec[:].unsqueeze(2).to_broadcast([P, H, DH]),
                            op=AluOp.mult)
                    st_sb = fpool.tile([P, 2, P], dt.float16, tag="st_sb")
                    tsc = dn_ps[:, 280:280 + P // 2].bitcast(dt.float16)
                    nc.tensor.transpose(tsc, s_sb[:, 0:P], ident16[:])
                    nc.scalar.copy(out=st_sb[:, 0, :], in_=tsc)
                    nc.tensor.transpose(tsc, s_sb[:, P:2 * P], ident16[:])
                    nc.scalar.copy(out=st_sb[:, 1, :], in_=tsc)
                    proj = dn_ps[:, 0:CDIM]
                    nc.tensor.matmul(proj, st_sb[:, 0, :],
                                     wproj[:, 0, :], start=True, stop=False,
                                     skip_group_check=True)
                    nc.tensor.matmul(proj, st_sb[:, 1, :],
                                     wproj[:, 1, :], start=False, stop=True,
                                     skip_group_check=True)
                    fa_t = fpool.tile([P, CDIM], dt.float16, tag="fa_t")
                    nc.sync.dma_start(out=fa_t[:], in_=FaRes_t[j * P:(j + 1) * P, :])
                    res = fpool.tile([P, CDIM], dt.float32, tag="res")
                    nc.vector.tensor_tensor(out=res[:], in0=proj,
                                            in1=fa_t[:], op=AluOp.add)
                    nc.sync.dma_start(out=OUT_t[j * P:(j + 1) * P, :], in_=res[:])

                def emit_S1(g):
                    """DMA + PE: KT, QeT matmuls for group g."""
                    j, g0, gn, last = g["key"]
                    if j not in blk:
                        Cj = int(CAP[j]) // P
                        c0 = int(coff[j])
                        fbet = gpool.tile([P, 2, CMAX * P], dt.float16, tag="fbet")
                        nc.sync.dma_start(out=fbet[:, :, :Cj * P],
                                          in_=FbET_t[:, :, c0:c0 + Cj * P])
                        selt = gpool.tile([P, CMAX * P], dt.float16, tag="selt")
                        nc.sync.dma_start(out=selt[:, :Cj * P],
                                          in_=SELT_t[:, c0:c0 + Cj * P])
                        sel = gpool.tile([P, CMAX * P], dt.float16, tag="sel")
                        nc.sync.dma_start(out=sel[:, :Cj * P],
                                          in_=SEL_t[:, c0:c0 + Cj * P])
                        # dn bank hosts [0:264] accum, [264:280] per-group
                        # logits, [280:408] finalize transpose scratch.
                        dn_ps = psDN.tile([P, 2 * CDIM], dt.float32, tag="dn")
                        blk[j] = {"fbet": fbet, "selt": selt, "sel": sel,
                                  "dn": dn_ps}
                    st = blk[j]
                    j_, g0_, gn_ = j, g0, gn
                    e0, e1 = g0_ * P, (g0_ + gn_) * P
                    ew = gn_ * P
                    # KT[d, e] per d-half u: accumulate over c-chunks t
                    kt_ps = psKT.tile([P, 2, G * P], dt.float32, tag="kt")
                    for u in range(2):
                        for t in range(2):
                            nc.tensor.matmul(kt_ps[:, u, :ew],
                                             wkt[:, t, u, :],
                                             st["fbet"][:, t, e0:e1],
                                             start=(t == 0), stop=(t == 1))
                    # QeT[d, e] per d-half u: Qblk.T @ one-hot
                    qt_ps = psQT.tile([P, 2, G * P], dt.float32, tag="qt")
                    for u in range(2):
                        nc.tensor.matmul(qt_ps[:, u, :ew],
                                         qres[:, j_, u * P:(u + 1) * P],
                                         st["selt"][:, e0:e1],
                                         start=True, stop=True)
                    g["kt"], g["qt"] = kt_ps, qt_ps

                def emit_S2(g):
                    """Scalar: QeT evac + exp; DVE: prod; PE: logits + V."""
                    j, g0, gn, last = g["key"]
                    ew = gn * P
                    qt_sb = wpool.tile([P, 2, G * P], dt.float16, tag="qt_sb")
                    nc.scalar.copy(out=qt_sb[:, :, :ew], in_=g["qt"][:, :, :ew])
                    prod = wpool.tile([P, 2, G * P], dt.float16, tag="prod")
                    with nc.allow_low_precision(reason="qk product fp16"):
                        nc.vector.tensor_tensor(
                            out=prod[:, :, :ew], in0=g["kt"][:, :, :ew],
                            in1=qt_sb[:, :, :ew], op=AluOp.mult)
                    # logits land in the (now dead) kt tile region; the RAW
                    # dep on prod already orders this after prod's read.
                    lg_ps = g["kt"][:, 0, 0:G * H].rearrange(
                        "p (g h) -> p g h", h=H)
                    for gg in range(gn):
                        for u in range(2):
                            nc.tensor.matmul(
                                lg_ps[:, gg, :],
                                prod[:, u, (gg * P):(gg + 1) * P],
                                hmap[:, u, :],
                                start=(u == 0), stop=(u == 1),
                                skip_group_check=True)
                    exwv = wpool.tile([P, G, H + CDIM], dt.float16, tag="exwv")
                    nc.scalar.activation(
                        out=exwv[:, :gn, 0:H], in_=lg_ps[:, :gn, :],
                        func=mybir.ActivationFunctionType.Exp, scale=SCALE)
                    # V[e, d] per tile: accumulate over c-chunks t
                    st = blk[j]
                    v_ps = psV.tile([P, G, CDIM], dt.float32, tag="v")
                    for gg in range(gn):
                        tt = g0 + gg
                        for t in range(2):
                            nc.tensor.matmul(v_ps[:, gg, :],
                                             st["fbet"][:, t, tt * P:(tt + 1) * P],
                                             wvt[:, t, :],
                                             start=(t == 0), stop=(t == 1))
                    g["exwv"], g["v"] = exwv, v_ps

                def emit_S3(g):
                    """DVE: exwv (broadcast exp); PE: accum (+fin)."""
                    j, g0, gn, last = g["key"]
                    Cj = int(CAP[j]) // P
                    exwv = g["exwv"]
                    with nc.allow_low_precision(reason="exp-weighted V fp16"):
                        nc.vector.tensor_tensor(
                            out=exwv[:, :gn, H:H + CDIM],
                            in0=exwv[:, :gn, 0:H].unsqueeze(3).to_broadcast(
                                [P, gn, H, DH]),
                            in1=g["v"][:, :gn, :], op=AluOp.mult)
                    st = blk[j]
                    for t in range(g0, g0 + gn):
                        nc.tensor.matmul(st["dn"][:, 0:H + CDIM],
                                         st["sel"][:, t * P:(t + 1) * P],
                                         exwv[:, t - g0, :],
                                         start=(t == 0), stop=(t == Cj - 1),
                                         skip_group_check=True)
                    if last:
                        emit_finalize(j, st["dn"])
                        del blk[j]

                pend = []
                for (j, g0, gn, last) in groups:
                    g = {"key": (j, g0, gn, last)}
                    emit_S1(g)
                    if len(pend) >= 2:
                        emit_S3(pend[-2])
                    if len(pend) >= 1:
                        emit_S2(pend[-1])
                    if len(pend) >= 2:
                        pend.pop(0)
                    pend.append(g)
                if pend:
                    emit_S2(pend[-1])
                    emit_S3(pend[0])
                    if len(pend) >= 2:
                        emit_S3(pend[1])

    nc.compile()
    return nc


TRACE = False          # set by test harness for NTFF profiling
LAST_RESULT = None     # BassKernelResults of the last run (for profiling)


def kernel(**inputs):
    global LAST_RESULT
    from concourse.bass_utils import run_bass_kernel_spmd

    meta, shared, per_core = preprocess(**inputs)
    nc = build_program(meta)
    in_maps = [dict(shared, **pc) for pc in per_core]
    res = run_bass_kernel_spmd(nc, in_maps, core_ids=list(range(NCORES)),
                               trace=TRACE)
    LAST_RESULT = res
    out = np.empty((NA, CDIM), F32)
    for m in range(NCORES):
        out[m * NAC:(m + 1) * NAC] = res.results[m]["OUT"][:NAC]
    return out


# revision 34
# speedup vs baseline: 1.1664x; 1.0558x over previous
"""Trainium2 Bass kernel for sparse multi-head edge attention (V7).

Computation (per the nn.Module):
    Q = Fa @ Wq.T, K = Fb @ Wk.T, V = Fb @ Wv.T   (H=8 heads x 32)
    per edge e: logit[e,h] = <Q[a_e,h,:], K[b_e,h,:]> / sqrt(32)
    segmented softmax over edges per query, out = Fa + (softmax(V)) @ Wproj.T

V7 strategy (vs the V3 baseline at ~1.04ms):
  - Queries sharded 8 ways; edges routed to the owner core, packed into
    128-edge tiles per 128-query block (identical host routing to V3).
  - K is recomputed per-edge *d-transposed* (KT[d, e] = Wk-row d . Fb[b_e])
    so the per-head logit reduction runs on the TENSOR engine as 4 tiny
    matmuls against a constant one-hot head map (HMAP), producing logits in
    normal [e, h] layout.  This removes the 265us tensor_reduce from DVE
    (tensor_reduce has no DVE fast mode) and keeps exp cheap (16 el/part).
  - Qe is selected transposed the same way (QeT = Qblk.T @ one-hot), so the
    QK product is one DVE multiply [128, 2, 256] -> fp16 SBUF.
  - V is recomputed per-edge in normal layout, evacuated PSUM->SBUF fp16 by
    the Scalar engine; exp values are broadcast-expanded across the head dim
    by GPSIMD (SBUF->SBUF); the exp-weighted V multiply then runs on DVE in
    2x_1p mode (all operands packed fp16 SBUF).
  - Accumulation of [den | num] per query stays a one-hot matmul per tile.
  - 3-stage software pipeline (emit S1(i), S2(i-1), S3(i-2)) so no in-order
    engine queue holds an op whose dependency chain crosses engines within
    the same group.
  All fp8 variants were numerically rejected (softmax amplifies the ~3%
  e4m3 quantization of K/V to >2e-2 output error); everything stays fp16
  with fp32 accumulation, matching the V3 numerics (max rel ~2.7e-3).
"""

import math

import numpy as np

P = 128
H = 8
DH = 32
CDIM = 256
NA = 50000
NB = 50000
NCORES = 8
NAC = NA // NCORES          # 6250 queries per core
NBLK = (NAC + P - 1) // P   # 49 query blocks per core
NPADQ = NBLK * P            # 6272 padded queries per core
CHUNK = 2048
G = 2                       # tiles per op-batching group
SCALE = 1.0 / math.sqrt(DH)

F16 = np.float16
F32 = np.float32


def _ceil128(x):
    return (np.asarray(x) + P - 1) // P * P


def preprocess(Fa, Fb, a_idx, b_idx, Wq, Wk, Wv, Wproj):
    """Host-side sharding: returns (meta, shared_inputs, per_core_inputs)."""
    a_idx = np.asarray(a_idx).astype(np.int64)
    b_idx = np.asarray(b_idx).astype(np.int64)
    Fa = np.asarray(Fa, F32)
    Fb = np.asarray(Fb, F32)

    # Degree-balanced query placement: deal queries (sorted by edge count,
    # serpentine order) across the 8*NBLK (core, block) cells so every
    # block's edge count is ~E/(8*NBLK) and the max-over-cores CAP padding
    # (6.1% with contiguous ranges) vanishes.
    ncell = NCORES * NBLK
    deg = np.bincount(a_idx, minlength=NA)
    order = np.argsort(-deg, kind="stable")
    rnk = np.arange(NA)
    rnd, pos = rnk // ncell, rnk % ncell
    cell_of_rank = np.where(rnd % 2 == 0, pos, ncell - 1 - pos)
    qcell = np.empty(NA, np.int64)
    qcell[order] = cell_of_rank
    qslot = np.empty(NA, np.int64)
    qslot[order] = rnd                      # one query per cell per round
    qcore = qcell // NBLK
    qblk = qcell % NBLK
    qpos = qblk * P + qslot                 # row within the core's padded Q

    core = qcore[a_idx]
    blk = qblk[a_idx]
    arel = qslot[a_idx]

    cnt = np.bincount(core * NBLK + blk, minlength=NCORES * NBLK)
    cnt = cnt.reshape(NCORES, NBLK)
    CAP = np.maximum(_ceil128(cnt.max(axis=0)), P)
    coff = np.concatenate([[0], np.cumsum(CAP)])
    TOT = int(coff[-1])

    # rank of each edge within its (core, blk) group
    ne = a_idx.shape[0]
    gid = core * NBLK + blk
    order = np.argsort(gid, kind="stable")
    counts = np.bincount(gid, minlength=NCORES * NBLK)
    gstart = np.concatenate([[0], np.cumsum(counts)])[:-1]
    rank = np.empty(ne, np.int64)
    rank[order] = np.arange(ne) - gstart[gid[order]]
    slot = coff[blk] + rank

    Fb16 = Fb.astype(F16)
    # HMAP: one-hot head map per d-half: HMAP[u][p, h] = 1 iff h == u*4 + p//DH
    hmap = np.zeros((P, 2, H), F16)
    for u in range(2):
        hmap[np.arange(P), u, u * 4 + np.arange(P) // DH] = 1.0
    # WKT: Wk.T arranged [c-chunk partitions, chunk, half, 128]:
    #   WKT[p, t, u, d] = Wk.T[t*128+p, u*128+d]
    wkT = Wk.T.astype(F16)  # [256 c, 256 d]
    WKT = wkT.reshape(2, P, 2, P).transpose(1, 0, 2, 3).copy()  # [p, t, u, d]
    # WVT: Wv.T arranged [c-chunk partitions, chunk, 256]
    wvT = Wv.T.astype(F16)
    WVT = wvT.reshape(2, P, CDIM).transpose(1, 0, 2).copy()     # [p, t, d]
    shared = {
        "WqT": Wq.T.astype(F16).copy(),
        "WKT": WKT,
        "WVT": WVT,
        "HMAP": hmap,
        "WprojT": Wproj.T.astype(F16).copy(),
        "IDENT16": np.eye(P, dtype=F16),
    }

    per_core = []
    for m in range(NCORES):
        msk = core == m
        sl = slot[msk]
        ar = arel[msk]
        fbe = np.zeros((TOT, CDIM), F16)
        fbe[sl] = Fb16[b_idx[msk]]
        selT = np.zeros((P, TOT), F16)
        selT[ar, sl] = 1.0
        sel = np.zeros((P, TOT), F16)
        sel[sl % P, (sl // P) * P + ar] = 1.0

        qmsk = qcore == m
        FaT = np.zeros((CDIM, NPADQ), F16)
        FaT[:, qpos[qmsk]] = Fa[qmsk].T.astype(F16)
        Fa_res = np.zeros((NPADQ, CDIM), F16)
        Fa_res[qpos[qmsk]] = Fa[qmsk].astype(F16)
        fbet2 = fbe.T.reshape(2, P, TOT).transpose(1, 0, 2)   # [128, 2, TOT]
        per_core.append({
            "FbET": np.ascontiguousarray(fbet2),
            "SELT": selT,
            "SEL": sel,
            "FaT": FaT,
            "FaRes": Fa_res,
        })

    meta = {"CAP": CAP.astype(int), "coff": coff.astype(int), "TOT": TOT,
            "qcore": qcore, "qpos": qpos}
    return meta, shared, per_core


def build_program(meta):
    import concourse.bacc as bacc
    import concourse.mybir as mybir
    from concourse.tile import TileContext

    dt = mybir.dt
    nc = bacc.Bacc("TRN2", target_bir_lowering=False, debug=False,
                   num_devices=NCORES)

    CAP, coff, TOT = meta["CAP"], meta["coff"], meta["TOT"]
    CMAX = int(CAP.max()) // P
    AluOp = mybir.AluOpType

    FbET_t = nc.dram_tensor("FbET", [P, 2, TOT], dt.float16, kind="ExternalInput")
    SELT_t = nc.dram_tensor("SELT", [P, TOT], dt.float16, kind="ExternalInput")
    SEL_t = nc.dram_tensor("SEL", [P, TOT], dt.float16, kind="ExternalInput")
    FaT_t = nc.dram_tensor("FaT", [CDIM, NPADQ], dt.float16, kind="ExternalInput")
    FaRes_t = nc.dram_tensor("FaRes", [NPADQ, CDIM], dt.float16, kind="ExternalInput")
    WqT_t = nc.dram_tensor("WqT", [CDIM, CDIM], dt.float16, kind="ExternalInput")
    WKT_t = nc.dram_tensor("WKT", [P, 2, 2, P], dt.float16, kind="ExternalInput")
    WVT_t = nc.dram_tensor("WVT", [P, 2, CDIM], dt.float16, kind="ExternalInput")
    HMAP_t = nc.dram_tensor("HMAP", [P, 2, H], dt.float16, kind="ExternalInput")
    WprojT_t = nc.dram_tensor("WprojT", [CDIM, CDIM], dt.float16, kind="ExternalInput")
    IDENT_t = nc.dram_tensor("IDENT16", [P, P], dt.float16, kind="ExternalInput")
    OUT_t = nc.dram_tensor("OUT", [NPADQ, CDIM], dt.float32, kind="ExternalOutput")

    with TileContext(nc) as tc:
        with tc.tile_pool(name="res", bufs=1) as rpool:
            wq = rpool.tile([P, 2, CDIM], dt.float16, tag="wq")
            wkt = rpool.tile([P, 2, 2, P], dt.float16, tag="wkt")
            wvt = rpool.tile([P, 2, CDIM], dt.float16, tag="wvt")
            hmap = rpool.tile([P, 2, H], dt.float16, tag="hmap")
            wproj = rpool.tile([P, 2, CDIM], dt.float16, tag="wproj")
            ident16 = rpool.tile([P, P], dt.float16, tag="ident16")
            nc.sync.dma_start(out=wq[:, 0, :], in_=WqT_t[0:P, :])
            nc.sync.dma_start(out=wq[:, 1, :], in_=WqT_t[P:2 * P, :])
            nc.sync.dma_start(out=wkt[:], in_=WKT_t[:, :, :, :])
            nc.sync.dma_start(out=wvt[:], in_=WVT_t[:, :, :])
            nc.sync.dma_start(out=hmap[:], in_=HMAP_t[:, :, :])
            nc.sync.dma_start(out=wproj[:, 0, :], in_=WprojT_t[0:P, :])
            nc.sync.dma_start(out=wproj[:, 1, :], in_=WprojT_t[P:2 * P, :])
            nc.sync.dma_start(out=ident16[:], in_=IDENT_t[:, :])
            qres = rpool.tile([P, NBLK, CDIM], dt.float16, tag="qres")

            # ---- Phase A: build Q into SBUF (Qblk[q, d] per block) ----
            with tc.tile_pool(name="bldA", bufs=2) as bpoolA, \
                 tc.tile_pool(name="psA", bufs=4, space="PSUM") as psA:
                for c0 in range(0, NPADQ, CHUNK):
                    nsub = min(CHUNK, NPADQ - c0) // P
                    ft = bpoolA.tile([P, 2, CHUNK], dt.float16, tag="ft")
                    nc.sync.dma_start(out=ft[:, 0, :nsub * P],
                                      in_=FaT_t[0:P, c0:c0 + nsub * P])
                    nc.sync.dma_start(out=ft[:, 1, :nsub * P],
                                      in_=FaT_t[P:2 * P, c0:c0 + nsub * P])
                    for s in range(nsub):
                        ps = psA.tile([P, CDIM], dt.float32, tag="psA")
                        nc.tensor.matmul(ps[:], ft[:, 0, s * P:(s + 1) * P],
                                         wq[:, 0, :], start=True, stop=False)
                        nc.tensor.matmul(ps[:], ft[:, 1, s * P:(s + 1) * P],
                                         wq[:, 1, :], start=False, stop=True)
                        jj = c0 // P + s
                        if jj % 2 == 0:
                            nc.scalar.copy(out=qres[:, jj, :], in_=ps[:])
                        else:
                            with nc.allow_low_precision(reason="q evac fp16"):
                                nc.vector.tensor_scalar_add(
                                    out=qres[:, jj, :], in0=ps[:],
                                    scalar1=0.0)

            # ---- Phase B: edge attention ----
            with tc.tile_pool(name="gat", bufs=4) as gpool, \
                 tc.tile_pool(name="wrk", bufs=3) as wpool, \
                 tc.tile_pool(name="fin", bufs=3) as fpool, \
                 tc.tile_pool(name="psKT", bufs=2, space="PSUM") as psKT, \
                 tc.tile_pool(name="psQT", bufs=2, space="PSUM") as psQT, \
                 tc.tile_pool(name="psV", bufs=2, space="PSUM") as psV, \
                 tc.tile_pool(name="psDN", bufs=2, space="PSUM") as psDN:
                groups = []
                for j in range(NBLK):
                    Cj = int(CAP[j]) // P
                    for g0 in range(0, Cj, G):
                        groups.append((j, g0, min(G, Cj - g0), g0 + G >= Cj))

                blk = {}        # j -> dict(dn, sel, fbet, selt)

                def emit_finalize(j, dn_ps):
                    # dn bank layout: [0:264] accum (dead after s_sb) ->
                    # reused for proj out [0:256]; [264:280] logits region;
                    # [280:408] transpose scratch (one half at a time).
                    den = fpool.tile([P, H], dt.float32, tag="den")
                    nc.vector.tensor_scalar_max(out=den[:], in0=dn_ps[:, 0:H],
                                                scalar1=1e-30)
                    rec = fpool.tile([P, H], dt.float32, tag="rec")
                    nc.vector.reciprocal(out=rec[:], in_=den[:])
                    s_sb = fpool.tile([P, CDIM], dt.float16, tag="s_sb")
                    with nc.allow_low_precision(reason="softmax out fp16"):
                        nc.vector.tensor_tensor(
                            out=s_sb[:], in0=dn_ps[:, H:H + CDIM],
                            in1=rec[:].unsqueeze(2).to_broadcast([P, H, DH]),
                            op=AluOp.mult)
                    st_sb = fpool.tile([P, 2, P], dt.float16, tag="st_sb")
                    tsc = dn_ps[:, 280:280 + P // 2].bitcast(dt.float16)
                    nc.tensor.transpose(tsc, s_sb[:, 0:P], ident16[:])
                    nc.scalar.copy(out=st_sb[:, 0, :], in_=tsc)
                    nc.tensor.transpose(tsc, s_sb[:, P:2 * P], ident16[:])
                    nc.scalar.copy(out=st_sb[:, 1, :], in_=tsc)
                    proj = dn_ps[:, 0:CDIM]
                    nc.tensor.matmul(proj, st_sb[:, 0, :],
                                     wproj[:, 0, :], start=True, stop=False,
                                     skip_group_check=True)
                    nc.tensor.matmul(proj, st_sb[:, 1, :],
                                     wproj[:, 1, :], start=False, stop=True,
                                     skip_group_check=True)
                    fa_t = fpool.tile([P, CDIM], dt.float16, tag="fa_t")
                    nc.sync.dma_start(out=fa_t[:], in_=FaRes_t[j * P:(j + 1) * P, :])
                    res = fpool.tile([P, CDIM], dt.float32, tag="res")
                    nc.vector.tensor_tensor(out=res[:], in0=proj,
                                            in1=fa_t[:], op=AluOp.add)
                    nc.sync.dma_start(out=OUT_t[j * P:(j + 1) * P, :], in_=res[:])

                def emit_S1(g):
                    """DMA + PE: KT, QeT matmuls for group g."""
                    j, g0, gn, last = g["key"]
                    if j not in blk:
                        Cj = int(CAP[j]) // P
                        c0 = int(coff[j])
                        fbet = gpool.tile([P, 2, CMAX * P], dt.float16, tag="fbet")
                        nc.sync.dma_start(out=fbet[:, :, :Cj * P],
                                          in_=FbET_t[:, :, c0:c0 + Cj * P])
                        selt = gpool.tile([P, CMAX * P], dt.float16, tag="selt")
                        nc.sync.dma_start(out=selt[:, :Cj * P],
                                          in_=SELT_t[:, c0:c0 + Cj * P])
                        sel = gpool.tile([P, CMAX * P], dt.float16, tag="sel")
                        nc.sync.dma_start(out=sel[:, :Cj * P],
                                          in_=SEL_t[:, c0:c0 + Cj * P])
                        # dn bank hosts [0:264] accum, [264:280] per-group
                        # logits, [280:408] finalize transpose scratch.
                        dn_ps = psDN.tile([P, 2 * CDIM], dt.float32, tag="dn")
                        blk[j] = {"fbet": fbet, "selt": selt, "sel": sel,
                                  "dn": dn_ps}
                    st = blk[j]
                    j_, g0_, gn_ = j, g0, gn
                    e0, e1 = g0_ * P, (g0_ + gn_) * P
                    ew = gn_ * P
                    # KT[d, e] per d-half u: accumulate over c-chunks t
                    kt_ps = psKT.tile([P, 2, G * P], dt.float32, tag="kt")
                    for u in range(2):
                        for t in range(2):
                            nc.tensor.matmul(kt_ps[:, u, :ew],
                                             wkt[:, t, u, :],
                                             st["fbet"][:, t, e0:e1],
                                             start=(t == 0), stop=(t == 1))
                    # QeT[d, e] per d-half u: Qblk.T @ one-hot
                    qt_ps = psQT.tile([P, 2, G * P], dt.float32, tag="qt")
                    for u in range(2):
                        nc.tensor.matmul(qt_ps[:, u, :ew],
                                         qres[:, j_, u * P:(u + 1) * P],
                                         st["selt"][:, e0:e1],
                                         start=True, stop=True)
                    g["kt"], g["qt"] = kt_ps, qt_ps

                def emit_S2(g):
                    """Scalar: QeT evac + exp; DVE: prod; PE: logits + V."""
                    j, g0, gn, last = g["key"]
                    ew = gn * P
                    qt_sb = wpool.tile([P, 2, G * P], dt.float16, tag="qt_sb")
                    nc.scalar.copy(out=qt_sb[:, :, :ew], in_=g["qt"][:, :, :ew])
                    prod = wpool.tile([P, 2, G * P], dt.float16, tag="prod")
                    with nc.allow_low_precision(reason="qk product fp16"):
                        nc.vector.tensor_tensor(
                            out=prod[:, :, :ew], in0=g["kt"][:, :, :ew],
                            in1=qt_sb[:, :, :ew], op=AluOp.mult)
                    # logits land in the (now dead) kt tile region; the RAW
                    # dep on prod already orders this after prod's read.
                    lg_ps = g["kt"][:, 0, 0:G * H].rearrange(
                        "p (g h) -> p g h", h=H)
                    for gg in range(gn):
                        for u in range(2):
                            nc.tensor.matmul(
                                lg_ps[:, gg, :],
                                prod[:, u, (gg * P):(gg + 1) * P],
                                hmap[:, u, :],
                                start=(u == 0), stop=(u == 1),
                                skip_group_check=True)
                    exwv = wpool.tile([P, G, H + CDIM], dt.float16, tag="exwv")
                    nc.scalar.activation(
                        out=exwv[:, :gn, 0:H], in_=lg_ps[:, :gn, :],
                        func=mybir.ActivationFunctionType.Exp, scale=SCALE)
                    # V[e, d] per tile: accumulate over c-chunks t
                    st = blk[j]
                    v_ps = psV.tile([P, G, CDIM], dt.float32, tag="v")
                    for gg in range(gn):
                        tt = g0 + gg
                        for t in range(2):
                            nc.tensor.matmul(v_ps[:, gg, :],
                                             st["fbet"][:, t, tt * P:(tt + 1) * P],
                                             wvt[:, t, :],
                                             start=(t == 0), stop=(t == 1))
                    g["exwv"], g["v"] = exwv, v_ps

                def emit_S3(g):
                    """DVE: exwv (broadcast exp); PE: accum (+fin)."""
                    j, g0, gn, last = g["key"]
                    Cj = int(CAP[j]) // P
                    exwv = g["exwv"]
                    with nc.allow_low_precision(reason="exp-weighted V fp16"):
                        nc.vector.tensor_tensor(
                            out=exwv[:, :gn, H:H + CDIM],
                            in0=exwv[:, :gn, 0:H].unsqueeze(3).to_broadcast(
                                [P, gn, H, DH]),
                            in1=g["v"][:, :gn, :], op=AluOp.mult)
                    st = blk[j]
                    for t in range(g0, g0 + gn):
                        nc.tensor.matmul(st["dn"][:, 0:H + CDIM],
                                         st["sel"][:, t * P:(t + 1) * P],
                                         exwv[:, t - g0, :],
                                         start=(t == 0), stop=(t == Cj - 1),
                                         skip_group_check=True)
                    if last:
                        emit_finalize(j, st["dn"])
                        del blk[j]

                pend = []
                for (j, g0, gn, last) in groups:
                    g = {"key": (j, g0, gn, last)}
                    emit_S1(g)
                    if len(pend) >= 2:
                        emit_S3(pend[-2])
                    if len(pend) >= 1:
                        emit_S2(pend[-1])
                    if len(pend) >= 2:
                        pend.pop(0)
                    pend.append(g)
                if pend:
                    emit_S2(pend[-1])
                    emit_S3(pend[0])
                    if len(pend) >= 2:
                        emit_S3(pend[1])

    nc.compile()
    return nc


TRACE = False          # set by test harness for NTFF profiling
LAST_RESULT = None     # BassKernelResults of the last run (for profiling)


def kernel(**inputs):
    global LAST_RESULT
    from concourse.bass_utils import run_bass_kernel_spmd

    meta, shared, per_core = preprocess(**inputs)
    nc = build_program(meta)
    in_maps = [dict(shared, **pc) for pc in per_core]
    res = run_bass_kernel_spmd(nc, in_maps, core_ids=list(range(NCORES)),
                               trace=TRACE)
    LAST_RESULT = res
    allout = np.stack([res.results[m]["OUT"] for m in range(NCORES)])
    return allout[meta["qcore"], meta["qpos"]].astype(F32)
